# revision 14
# baseline (speedup 1.0000x reference)
"""MoE (top-2 of 8 experts, SwiGLU) kernel for 8 TRN2 NeuronCores.

Expert-parallel, collective-free. Core e holds expert e's weights resident
in SBUF and computes y_t = MLP_e(x_t) * w[e,t] for exactly the tokens
routed to e (host-side gather builds hsTg = hs^T restricted to those
tokens; pad columns are zero with zero combine weight). Each core writes
its [C_pad, H] block of wcg-scaled rows; the host assembly scatter-adds
the two expert contributions per token (16.7 MFLOP, 0.08% of the matmul
work — measured on-device AllToAll combines cost +100-230us of pure
latency because the software collective degrades concurrent compute, far
more than this pointwise add is worth).

All addressing is compile-time and identical across cores (C_pad =
pad128(max tokens per expert)); per-core variation lives in the data.

Gate/up matmuls stream 512 columns wide (PSUM-bank max; measured optimal
~0.53ns/row on HW). The <512-column tail block interleaves the gate/up
accumulation chains to dodge the ~300ns narrow-matmul latency floor.
Weight loads are split into h-quarters and interleaved with the first
block's chains so the PE starts ~12us in. Matmul operands are bf16 (fp32
PSUM accumulation); rel err vs the fp32 reference is ~5e-3.
"""

import numpy as np
import ml_dtypes

import jax
import concourse.bass as bass
import concourse.tile as tile
from concourse import bacc, mybir
from concourse.bass import ts

E, H, I, T, KTOP = 8, 2048, 1408, 4096, 2
NCORES = 8

BF16 = mybir.dt.bfloat16
F32 = mybir.dt.float32


def _ceil128(x):
    return (x + 127) // 128 * 128


def _route(hidden_states, top_k_index, top_k_weights):
    """Host-side routing. Returns per-core in_maps (hsTg, wcg), C_pad, and
    per-core token lists for host assembly."""
    hs = np.asarray(hidden_states, dtype=np.float32)
    idx = np.asarray(top_k_index).astype(np.int64)
    tw = np.asarray(top_k_weights, dtype=np.float32)

    w = np.zeros((E, T), dtype=np.float32)
    tarange = np.arange(T)
    for k in range(KTOP):
        np.add.at(w, (idx[:, k], tarange), tw[:, k])

    toks = [np.where(w[e] > 0)[0] for e in range(E)]
    # a token with both top-k slots on one expert still appears once, with
    # the summed weight; w>0 holds a.s. for uniform(0,1) weights
    C_pad = _ceil128(max(len(t) for t in toks))

    hsT_bf = np.ascontiguousarray(hs.T).astype(ml_dtypes.bfloat16)
    in_maps, plans = [], []
    for e in range(E):
        n = len(toks[e])
        cols = np.zeros(C_pad, dtype=np.int64)
        cols[:n] = toks[e]
        wcg = np.zeros(C_pad, dtype=np.float32)
        wcg[:n] = w[e, toks[e]]
        g = hsT_bf[:, cols]
        g[:, n:] = 0
        in_maps.append({"hsTg": np.ascontiguousarray(g), "wcg": wcg})
        plans.append(toks[e])
    return in_maps, C_pad, plans


def _build_moe(C_pad, h=H, i_sz=I, ncores=NCORES):
    hc, ic2 = h // 128, i_sz // 128
    hh = hc // 4  # h-chunk quarter for interleaved weight loads
    ntiles = C_pad // 128

    blocks = []
    pos = 0
    while C_pad - pos > 512:
        blocks.append((pos, 512))
        pos += 512
    if C_pad - pos:
        blocks.append((pos, C_pad - pos))

    nc = bacc.Bacc("TRN2", target_bir_lowering=False, debug=False,
                   num_devices=ncores)
    hsTg = nc.declare_dram_parameter("hsTg", [h, C_pad], BF16, isOutput=False).ap()
    wg = nc.declare_dram_parameter("wg", [h, i_sz], BF16, isOutput=False).ap()
    wu = nc.declare_dram_parameter("wu", [h, i_sz], BF16, isOutput=False).ap()
    wd = nc.declare_dram_parameter("wd", [i_sz, h], BF16, isOutput=False).ap()
    wcg = nc.declare_dram_parameter("wcg", [C_pad], F32, isOutput=False).ap()
    out = nc.declare_dram_parameter("out", [C_pad, h], BF16, isOutput=True).ap()

    silu = mybir.ActivationFunctionType.Silu

    with tile.TileContext(nc) as tc:
        with (
            tc.tile_pool(name="wpool", bufs=1) as wpool,
            tc.tile_pool(name="hspool", bufs=2) as hspool,
            tc.tile_pool(name="apool", bufs=1) as apool,
            tc.tile_pool(name="stage", bufs=2) as stage,
            tc.tile_pool(name="ypool", bufs=3) as ypool,
            tc.tile_pool(name="pg", bufs=2, space="PSUM") as pg,
            tc.tile_pool(name="pu", bufs=2, space="PSUM") as pu,
            tc.tile_pool(name="py", bufs=4, space="PSUM") as py,
        ):
            # hidden states and weights are loaded in h-quarters, interleaved
            # in the exact order the first gate chain consumes them (hs q0 +
            # wg q0 first, 0.5+1.4MB), so the PE starts ~2us in instead of
            # waiting for the full 3.5MB+.
            def load_hs_quarters(pos, nb):
                tiles = []
                for q in range(4):
                    t = hspool.tile([128, hh, nb], BF16, tag=f"hsq{q}")
                    tiles.append(t)
                return tiles

            def dma_hs_quarter(tiles, q, pos, nb):
                nc.sync.dma_start(
                    out=tiles[q][:],
                    in_=hsTg[q * hh * 128:(q + 1) * hh * 128, pos:pos + nb]
                    .rearrange("(c p) t -> p c t", p=128))

            wg_h = [wpool.tile([128, hh, i_sz], BF16, name=f"wg{i}",
                               tag=f"wg{i}") for i in range(4)]
            wu_h = [wpool.tile([128, hh, i_sz], BF16, name=f"wu{i}",
                               tag=f"wu{i}") for i in range(4)]
            (pos0, nb0) = blocks[0]
            hs0 = load_hs_quarters(pos0, nb0)
            for i in range(4):
                dma_hs_quarter(hs0, i, pos0, nb0)
                nc.sync.dma_start(
                    out=wg_h[i][:],
                    in_=wg[i * hh * 128:(i + 1) * hh * 128, :]
                    .rearrange("(c p) i -> p c i", p=128))
            for i in range(4):
                nc.sync.dma_start(
                    out=wu_h[i][:],
                    in_=wu[i * hh * 128:(i + 1) * hh * 128, :]
                    .rearrange("(c p) i -> p c i", p=128))
            wd_sb = wpool.tile([128, ic2, h], BF16, tag="wd")
            nc.sync.dma_start(out=wd_sb[:], in_=wd.rearrange("(c p) j -> p c j", p=128))
            wcg_sb = wpool.tile([128, ntiles], F32, tag="wcg")
            nc.sync.dma_start(out=wcg_sb[:], in_=wcg.rearrange("(ct p) -> p ct", p=128))

            for bi, (pos, nb) in enumerate(blocks):
                if bi == 0:
                    hs_t = hs0
                else:
                    hs_t = load_hs_quarters(pos, nb)
                    for q in range(4):
                        dma_hs_quarter(hs_t, q, pos, nb)

                aT = apool.tile([128, ic2, nb], BF16, tag="aT")
                interleave = nb < 512
                for it in range(ic2):
                    psg = pg.tile([128, nb], F32, tag="psg")
                    psu = pu.tile([128, nb], F32, tag="psu")
                    if interleave:
                        for c in range(hc):
                            half, cc = c // hh, c % hh
                            nc.tensor.matmul(
                                psg[:], lhsT=wg_h[half][:, cc, ts(it, 128)],
                                rhs=hs_t[half][:, cc, :],
                                start=(c == 0), stop=(c == hc - 1))
                            nc.tensor.matmul(
                                psu[:], lhsT=wu_h[half][:, cc, ts(it, 128)],
                                rhs=hs_t[half][:, cc, :],
                                start=(c == 0), stop=(c == hc - 1))
                    else:
                        for half in range(4):
                            for cc in range(hh):
                                c = half * hh + cc
                                nc.tensor.matmul(
                                    psg[:], lhsT=wg_h[half][:, cc, ts(it, 128)],
                                    rhs=hs_t[half][:, cc, :],
                                    start=(c == 0), stop=(c == hc - 1))
                            for cc in range(hh):
                                c = half * hh + cc
                                nc.tensor.matmul(
                                    psu[:], lhsT=wu_h[half][:, cc, ts(it, 128)],
                                    rhs=hs_t[half][:, cc, :],
                                    start=(c == 0), stop=(c == hc - 1))
                    sil = stage.tile([128, nb], F32, tag="sil")
                    nc.scalar.activation(out=sil[:], in_=psg[:], func=silu)
                    nc.vector.tensor_mul(aT[:, it, :], sil[:], psu[:])

                for ct in range(nb // 128):
                    gct = pos // 128 + ct
                    g0 = gct * 128
                    y_sb = ypool.tile([128, h], BF16, tag="ysb")
                    for hb in range(h // 512):
                        psy = py.tile([128, 512], F32, tag="psy")
                        for c2 in range(ic2):
                            nc.tensor.matmul(psy[:],
                                             lhsT=aT[:, c2, ts(ct, 128)],
                                             rhs=wd_sb[:, c2, ts(hb, 512)],
                                             start=(c2 == 0),
                                             stop=(c2 == ic2 - 1))
                        nc.vector.tensor_scalar_mul(
                            y_sb[:, ts(hb, 512)], psy[:],
                            wcg_sb[:, gct:gct + 1])
                    nc.sync.dma_start(out=out[g0:g0 + 128, :], in_=y_sb[:])

    nc.compile()
    return nc


class _Runner:
    """Compile once, execute many. Mirrors bass2jax.run_bass_via_pjrt's
    multi-core path but keeps the jitted callable (and device-resident
    inputs) alive so repeat executions skip XLA/NEFF compilation."""

    def __init__(self, nc, n_cores):
        from concourse import bass2jax, mybir as _mybir
        from jax.experimental.shard_map import shard_map
        from jax.sharding import Mesh, PartitionSpec

        bass2jax.install_neuronx_cc_hook()
        partition_name = (nc.partition_id_tensor.name
                          if nc.partition_id_tensor else None)

        in_names, out_names, out_avals, zero_outs = [], [], [], []
        for alloc in nc.m.functions[0].allocations:
            if not isinstance(alloc, _mybir.MemoryLocationSet):
                continue
            name = alloc.memorylocations[0].name
            if alloc.kind == "ExternalInput":
                if name != partition_name:
                    in_names.append(name)
            elif alloc.kind == "ExternalOutput":
                shape = tuple(alloc.tensor_shape)
                dtype = _mybir.dt.np(alloc.dtype)
                out_names.append(name)
                out_avals.append(jax.core.ShapedArray(shape, dtype))
                zero_outs.append(np.zeros(shape, dtype))
        self.n_params = len(in_names)
        self.param_names = list(in_names)
        self.out_names = out_names
        self.out_avals = out_avals
        self.n_cores = n_cores
        all_names = in_names + out_names
        if partition_name is not None:
            all_names.append(partition_name)

        def _body(*args):
            operands = list(args)
            if partition_name is not None:
                operands.append(bass2jax.partition_id_tensor())
            outs = bass2jax._bass_exec_p.bind(
                *operands,
                out_avals=tuple(out_avals),
                in_names=tuple(all_names),
                out_names=tuple(out_names),
                lowering_input_output_aliases=(),
                sim_require_finite=True,
                sim_require_nnan=True,
                nc=nc,
            )
            return tuple(outs)

        devices = jax.devices()[:n_cores]
        assert len(devices) == n_cores
        mesh = Mesh(np.asarray(devices), ("core",))
        n_ops = self.n_params + len(out_names)
        self._body = _body
        self._mesh = mesh
        self._in_specs = (PartitionSpec("core"),) * n_ops
        self._out_specs = (PartitionSpec("core"),) * len(out_names)
        self._fn = jax.jit(
            shard_map(_body, mesh=mesh,
                      in_specs=self._in_specs,
                      out_specs=self._out_specs,
                      check_rep=False),
            keep_unused=True)
        self._zeros = [
            np.zeros((n_cores * z.shape[0], *z.shape[1:]), z.dtype)
            for z in zero_outs
        ]
        self._dev_args = None

    def prepare(self, in_maps):
        """Stage concatenated inputs, sharded across cores so execution
        never reshards (resharding would ship bytes through the host)."""
        from jax.sharding import NamedSharding, PartitionSpec
        sh = NamedSharding(self._mesh, PartitionSpec("core"))
        concat = [
            np.concatenate([np.asarray(in_maps[c][name])
                            for c in range(self.n_cores)], axis=0)
            for name in self.param_names
        ]
        self._dev_args = [jax.device_put(a, sh) for a in concat + self._zeros]

    def execute(self):
        outs = self._fn(*self._dev_args)
        jax.block_until_ready(outs)
        return outs

    def execute_chain(self, k):
        """Issue k async executions back-to-back, block once at the end.
        Device-side queuing overlaps the per-dispatch host round-trip, so
        wall(k) = floor + k * hw_exec and the slope over k isolates
        hw_exec."""
        outs = None
        for _ in range(k):
            outs = self._fn(*self._dev_args)
        jax.block_until_ready(outs)
        return outs

    def run(self, in_maps):
        self.prepare(in_maps)
        outs = self.execute()
        return [
            {name: np.asarray(outs[i]).reshape(self.n_cores,
                                               *self.out_avals[i].shape)[c]
             for i, name in enumerate(self.out_names)}
            for c in range(self.n_cores)
        ]


def _assemble(results, plans):
    full = np.zeros((T, H), dtype=np.float32)
    for e in range(NCORES):
        L = plans[e]
        if len(L):
            r_out = np.asarray(results[e]["out"][:len(L)], dtype=np.float32)
            full[L] += r_out
    return full


_RUNNERS = {}


def _get_runner(C_pad):
    if C_pad not in _RUNNERS:
        nc = _build_moe(C_pad)
        _RUNNERS[C_pad] = _Runner(nc, NCORES)
    return _RUNNERS[C_pad]


def kernel(hidden_states, top_k_index, top_k_weights, Wg, Wu, Wd):
    in_maps, C_pad, plans = _route(hidden_states, top_k_index, top_k_weights)
    for e in range(E):
        in_maps[e]["wg"] = np.asarray(Wg[e], dtype=np.float32).astype(ml_dtypes.bfloat16)
        in_maps[e]["wu"] = np.asarray(Wu[e], dtype=np.float32).astype(ml_dtypes.bfloat16)
        in_maps[e]["wd"] = np.asarray(Wd[e], dtype=np.float32).astype(ml_dtypes.bfloat16)
    runner = _get_runner(C_pad)
    results = runner.run(in_maps)
    return _assemble(results, plans)


# revision 17
# speedup vs baseline: 1.0381x; 1.0381x over previous
"""MoE (top-2 of 8 experts, SwiGLU) kernel for 8 TRN2 NeuronCores.

Expert-parallel, collective-free. Core e holds expert e's weights resident
in SBUF and computes y_t = MLP_e(x_t) * w[e,t] for exactly the tokens
routed to e (host-side gather builds hsTg = hs^T restricted to those
tokens; pad columns are zero with zero combine weight). Each core writes
its [C_pad, H] block of wcg-scaled rows; the host assembly scatter-adds
the two expert contributions per token (16.7 MFLOP, 0.08% of the matmul
work — measured on-device AllToAll combines cost +100-230us of pure
latency because the software collective degrades concurrent compute, far
more than this pointwise add is worth).

All addressing is compile-time and identical across cores (C_pad =
pad128(max tokens per expert)); per-core variation lives in the data.

Gate/up matmuls stream 512 columns wide (PSUM-bank max; measured optimal
~0.53ns/row on HW). The <512-column tail block interleaves the gate/up
accumulation chains to dodge the ~300ns narrow-matmul latency floor.
Weight loads are split into h-quarters and interleaved with the first
block's chains so the PE starts ~12us in. Matmul operands are bf16 (fp32
PSUM accumulation); rel err vs the fp32 reference is ~5e-3.
"""

import numpy as np
import ml_dtypes

import jax
import concourse.bass as bass
import concourse.tile as tile
from concourse import bacc, mybir
from concourse.bass import ts

E, H, I, T, KTOP = 8, 2048, 1408, 4096, 2
NCORES = 8

BF16 = mybir.dt.bfloat16
F32 = mybir.dt.float32


def _ceil128(x):
    return (x + 127) // 128 * 128


def _route(hidden_states, top_k_index, top_k_weights):
    """Host-side routing. Returns per-core in_maps (hsTg, wcg), C_pad, and
    per-core token lists for host assembly."""
    hs = np.asarray(hidden_states, dtype=np.float32)
    idx = np.asarray(top_k_index).astype(np.int64)
    tw = np.asarray(top_k_weights, dtype=np.float32)

    w = np.zeros((E, T), dtype=np.float32)
    tarange = np.arange(T)
    for k in range(KTOP):
        np.add.at(w, (idx[:, k], tarange), tw[:, k])

    toks = [np.where(w[e] > 0)[0] for e in range(E)]
    # a token with both top-k slots on one expert still appears once, with
    # the summed weight; w>0 holds a.s. for uniform(0,1) weights
    C_pad = _ceil128(max(len(t) for t in toks))

    hsT_bf = np.ascontiguousarray(hs.T).astype(ml_dtypes.bfloat16)
    in_maps, plans = [], []
    for e in range(E):
        n = len(toks[e])
        cols = np.zeros(C_pad, dtype=np.int64)
        cols[:n] = toks[e]
        wcg = np.zeros(C_pad, dtype=np.float32)
        wcg[:n] = w[e, toks[e]]
        g = hsT_bf[:, cols]
        g[:, n:] = 0
        in_maps.append({"hsTg": np.ascontiguousarray(g), "wcg": wcg})
        plans.append(toks[e])
    return in_maps, C_pad, plans


def _build_moe(C_pad, h=H, i_sz=I, ncores=NCORES):
    hc, ic2 = h // 128, i_sz // 128
    hh = hc // 4  # h-chunk quarter for interleaved weight loads
    ntiles = C_pad // 128

    blocks = []
    pos = 0
    while C_pad - pos > 512:
        blocks.append((pos, 512))
        pos += 512
    if C_pad - pos:
        blocks.append((pos, C_pad - pos))

    nc = bacc.Bacc("TRN2", target_bir_lowering=False, debug=False,
                   num_devices=ncores)
    hsTg = nc.declare_dram_parameter("hsTg", [h, C_pad], BF16, isOutput=False).ap()
    # wg|wu|wd raveled into one parameter: fewer NEFF inputs per exec
    w3 = nc.declare_dram_parameter("w3", [3 * h * i_sz], BF16, isOutput=False).ap()
    wcg = nc.declare_dram_parameter("wcg", [C_pad], F32, isOutput=False).ap()
    out = nc.declare_dram_parameter("out", [C_pad, h], BF16, isOutput=True).ap()
    qsz = hh * 128 * i_sz  # one h-quarter of wg/wu, raveled

    silu = mybir.ActivationFunctionType.Silu

    with tile.TileContext(nc) as tc:
        with (
            tc.tile_pool(name="wpool", bufs=1) as wpool,
            tc.tile_pool(name="hspool", bufs=2) as hspool,
            tc.tile_pool(name="apool", bufs=1) as apool,
            tc.tile_pool(name="stage", bufs=2) as stage,
            tc.tile_pool(name="ypool", bufs=3) as ypool,
            tc.tile_pool(name="pg", bufs=2, space="PSUM") as pg,
            tc.tile_pool(name="pu", bufs=2, space="PSUM") as pu,
            tc.tile_pool(name="py", bufs=4, space="PSUM") as py,
        ):
            # hidden states and weights are loaded in h-quarters, interleaved
            # in the exact order the first gate chain consumes them (hs q0 +
            # wg q0 first, 0.5+1.4MB), so the PE starts ~2us in instead of
            # waiting for the full 3.5MB+.
            def load_hs_quarters(pos, nb):
                tiles = []
                for q in range(4):
                    t = hspool.tile([128, hh, nb], BF16, tag=f"hsq{q}")
                    tiles.append(t)
                return tiles

            def dma_hs_quarter(tiles, q, pos, nb):
                nc.sync.dma_start(
                    out=tiles[q][:],
                    in_=hsTg[q * hh * 128:(q + 1) * hh * 128, pos:pos + nb]
                    .rearrange("(c p) t -> p c t", p=128))

            wg_h = [wpool.tile([128, hh, i_sz], BF16, name=f"wg{i}",
                               tag=f"wg{i}") for i in range(4)]
            wu_h = [wpool.tile([128, hh, i_sz], BF16, name=f"wu{i}",
                               tag=f"wu{i}") for i in range(4)]
            (pos0, nb0) = blocks[0]
            hs0 = load_hs_quarters(pos0, nb0)
            for i in range(4):
                dma_hs_quarter(hs0, i, pos0, nb0)
                nc.sync.dma_start(
                    out=wg_h[i][:],
                    in_=w3[i * qsz:(i + 1) * qsz]
                    .rearrange("(c p i) -> p c i", p=128, i=i_sz))
            for i in range(4):
                nc.sync.dma_start(
                    out=wu_h[i][:],
                    in_=w3[h * i_sz + i * qsz:h * i_sz + (i + 1) * qsz]
                    .rearrange("(c p i) -> p c i", p=128, i=i_sz))
            wd_sb = wpool.tile([128, ic2, h], BF16, tag="wd")
            nc.sync.dma_start(
                out=wd_sb[:],
                in_=w3[2 * h * i_sz:3 * h * i_sz]
                .rearrange("(c p j) -> p c j", p=128, j=h))
            wcg_sb = wpool.tile([128, ntiles], F32, tag="wcg")
            nc.sync.dma_start(out=wcg_sb[:], in_=wcg.rearrange("(ct p) -> p ct", p=128))

            for bi, (pos, nb) in enumerate(blocks):
                if bi == 0:
                    hs_t = hs0
                else:
                    hs_t = load_hs_quarters(pos, nb)
                    for q in range(4):
                        dma_hs_quarter(hs_t, q, pos, nb)

                aT = apool.tile([128, ic2, nb], BF16, tag="aT")
                interleave = nb < 512
                for it in range(ic2):
                    psg = pg.tile([128, nb], F32, tag="psg")
                    psu = pu.tile([128, nb], F32, tag="psu")
                    if interleave:
                        for c in range(hc):
                            half, cc = c // hh, c % hh
                            nc.tensor.matmul(
                                psg[:], lhsT=wg_h[half][:, cc, ts(it, 128)],
                                rhs=hs_t[half][:, cc, :],
                                start=(c == 0), stop=(c == hc - 1))
                            nc.tensor.matmul(
                                psu[:], lhsT=wu_h[half][:, cc, ts(it, 128)],
                                rhs=hs_t[half][:, cc, :],
                                start=(c == 0), stop=(c == hc - 1))
                    else:
                        for half in range(4):
                            for cc in range(hh):
                                c = half * hh + cc
                                nc.tensor.matmul(
                                    psg[:], lhsT=wg_h[half][:, cc, ts(it, 128)],
                                    rhs=hs_t[half][:, cc, :],
                                    start=(c == 0), stop=(c == hc - 1))
                            for cc in range(hh):
                                c = half * hh + cc
                                nc.tensor.matmul(
                                    psu[:], lhsT=wu_h[half][:, cc, ts(it, 128)],
                                    rhs=hs_t[half][:, cc, :],
                                    start=(c == 0), stop=(c == hc - 1))
                    sil = stage.tile([128, nb], F32, tag="sil")
                    nc.scalar.activation(out=sil[:], in_=psg[:], func=silu)
                    nc.vector.tensor_mul(aT[:, it, :], sil[:], psu[:])

                for ct in range(nb // 128):
                    gct = pos // 128 + ct
                    g0 = gct * 128
                    y_sb = ypool.tile([128, h], BF16, tag="ysb")
                    for hb in range(h // 512):
                        psy = py.tile([128, 512], F32, tag="psy")
                        for c2 in range(ic2):
                            nc.tensor.matmul(psy[:],
                                             lhsT=aT[:, c2, ts(ct, 128)],
                                             rhs=wd_sb[:, c2, ts(hb, 512)],
                                             start=(c2 == 0),
                                             stop=(c2 == ic2 - 1))
                        nc.vector.tensor_scalar_mul(
                            y_sb[:, ts(hb, 512)], psy[:],
                            wcg_sb[:, gct:gct + 1])
                    nc.sync.dma_start(out=out[g0:g0 + 128, :], in_=y_sb[:])

    nc.compile()
    return nc


class _Runner:
    """Compile once, execute many. Mirrors bass2jax.run_bass_via_pjrt's
    multi-core path but keeps the jitted callable (and device-resident
    inputs) alive so repeat executions skip XLA/NEFF compilation."""

    def __init__(self, nc, n_cores):
        from concourse import bass2jax, mybir as _mybir
        from jax.experimental.shard_map import shard_map
        from jax.sharding import Mesh, PartitionSpec

        bass2jax.install_neuronx_cc_hook()
        partition_name = (nc.partition_id_tensor.name
                          if nc.partition_id_tensor else None)

        in_names, out_names, out_avals, zero_outs = [], [], [], []
        for alloc in nc.m.functions[0].allocations:
            if not isinstance(alloc, _mybir.MemoryLocationSet):
                continue
            name = alloc.memorylocations[0].name
            if alloc.kind == "ExternalInput":
                if name != partition_name:
                    in_names.append(name)
            elif alloc.kind == "ExternalOutput":
                shape = tuple(alloc.tensor_shape)
                dtype = _mybir.dt.np(alloc.dtype)
                out_names.append(name)
                out_avals.append(jax.core.ShapedArray(shape, dtype))
                zero_outs.append(np.zeros(shape, dtype))
        self.n_params = len(in_names)
        self.param_names = list(in_names)
        self.out_names = out_names
        self.out_avals = out_avals
        self.n_cores = n_cores
        all_names = in_names + out_names
        if partition_name is not None:
            all_names.append(partition_name)

        def _body(*args):
            operands = list(args)
            if partition_name is not None:
                operands.append(bass2jax.partition_id_tensor())
            outs = bass2jax._bass_exec_p.bind(
                *operands,
                out_avals=tuple(out_avals),
                in_names=tuple(all_names),
                out_names=tuple(out_names),
                lowering_input_output_aliases=(),
                sim_require_finite=True,
                sim_require_nnan=True,
                nc=nc,
            )
            return tuple(outs)

        devices = jax.devices()[:n_cores]
        assert len(devices) == n_cores
        mesh = Mesh(np.asarray(devices), ("core",))
        n_ops = self.n_params + len(out_names)
        self._body = _body
        self._mesh = mesh
        self._in_specs = (PartitionSpec("core"),) * n_ops
        self._out_specs = (PartitionSpec("core"),) * len(out_names)
        self._fn = jax.jit(
            shard_map(_body, mesh=mesh,
                      in_specs=self._in_specs,
                      out_specs=self._out_specs,
                      check_rep=False),
            keep_unused=True)
        self._zeros = [
            np.zeros((n_cores * z.shape[0], *z.shape[1:]), z.dtype)
            for z in zero_outs
        ]
        self._dev_args = None

    def prepare(self, in_maps):
        """Stage concatenated inputs, sharded across cores so execution
        never reshards (resharding would ship bytes through the host)."""
        from jax.sharding import NamedSharding, PartitionSpec
        sh = NamedSharding(self._mesh, PartitionSpec("core"))
        concat = [
            np.concatenate([np.asarray(in_maps[c][name])
                            for c in range(self.n_cores)], axis=0)
            for name in self.param_names
        ]
        self._dev_args = [jax.device_put(a, sh) for a in concat + self._zeros]

    def execute(self):
        outs = self._fn(*self._dev_args)
        jax.block_until_ready(outs)
        return outs

    def execute_chain(self, k):
        """Issue k async executions back-to-back, block once at the end.
        Device-side queuing overlaps the per-dispatch host round-trip, so
        wall(k) = floor + k * hw_exec and the slope over k isolates
        hw_exec."""
        outs = None
        for _ in range(k):
            outs = self._fn(*self._dev_args)
        jax.block_until_ready(outs)
        return outs

    def run(self, in_maps):
        self.prepare(in_maps)
        outs = self.execute()
        return [
            {name: np.asarray(outs[i]).reshape(self.n_cores,
                                               *self.out_avals[i].shape)[c]
             for i, name in enumerate(self.out_names)}
            for c in range(self.n_cores)
        ]


def _assemble(results, plans):
    full = np.zeros((T, H), dtype=np.float32)
    for e in range(NCORES):
        L = plans[e]
        if len(L):
            r_out = np.asarray(results[e]["out"][:len(L)], dtype=np.float32)
            full[L] += r_out
    return full


_RUNNERS = {}


def _get_runner(C_pad):
    if C_pad not in _RUNNERS:
        nc = _build_moe(C_pad)
        _RUNNERS[C_pad] = _Runner(nc, NCORES)
    return _RUNNERS[C_pad]


def kernel(hidden_states, top_k_index, top_k_weights, Wg, Wu, Wd):
    in_maps, C_pad, plans = _route(hidden_states, top_k_index, top_k_weights)
    for e in range(E):
        in_maps[e]["w3"] = np.concatenate([
            np.asarray(Wg[e], dtype=np.float32).astype(ml_dtypes.bfloat16).ravel(),
            np.asarray(Wu[e], dtype=np.float32).astype(ml_dtypes.bfloat16).ravel(),
            np.asarray(Wd[e], dtype=np.float32).astype(ml_dtypes.bfloat16).ravel(),
        ])
    runner = _get_runner(C_pad)
    results = runner.run(in_maps)
    return _assemble(results, plans)
